# revision 5
# baseline (speedup 1.0000x reference)
"""Trainium2 Bass kernel for the 2-layer LSTM language model — v2.

Strategy: 8-way tensor parallelism over the hidden/gate dimension, with the
per-step hidden-state exchange done by direct SBUF->SBUF remote DMA
broadcasts (SWDGE remote_dma_broadcast) instead of ncfw AllGather
collectives, and the exchange split in two (h0 right after layer 0's cell,
h1 after layer 1's) so each transfer overlaps the remaining matmuls.

- Core k owns hidden slice k (128 of 1024 units) of both LSTM layers:
  it computes gate columns [f_k|i_k|o_k|t_k] (512 of 4096) each step.
- Layer 1 lags layer 0 by one step.  Per step t (period p):
    tensor: z0x(t) | z0h(t) | TP1(h1(p-2)) | z1h0(t-1)+b1 | TP0(t) | z1h1(t-1)
  After z0h: act0/cell0 -> h0l -> TP0 -> copy -> bc0 (broadcast h0T(t) 32KB
  to all 8 cores' gath0 ring slot).  After z1: act1/cell1 -> h1l; its
  transpose+copy+bc1 happen early in period t+2.
- Every core receives every h1T tile each step; the gathered [128, 1024]
  slab is DMAed to private DRAM (h1full), so the output MLP needs no
  final collective.  Output MLP identical to v1 (transposed hs, per-
  partition relu bias, 4-step groups).
- The embedding, the speaker-flag rank-1 term, and b0 are folded into a
  single [384, 4096] input-side weight on the host.
Matmul operands are bf16 (f32 PSUM accumulation); cell state stays f32.
"""
import numpy as np
import ml_dtypes

import concourse.bass as bass
import concourse.mybir as mybir
from concourse import library_config
from concourse.bass_utils import run_bass_kernel_spmd
from concourse.library_overlay import lower_extended_insts

BF16 = ml_dtypes.bfloat16

T_FULL, B, IND = 512, 128, 259
EMB, NN, VOCAB, BIG = 512, 1024, 256, 128
NC = 8
SL = NN // NC          # 128 hidden units per core
GC = 4 * SL            # 512 gate columns per core
KP = 384               # padded inpT rows = 3 K-tiles (259 data + 1 + s + pad)
NSLOT = 4              # gather ring slots
AF = mybir.dt.ActivationFunctionType if hasattr(mybir.dt, "ActivationFunctionType") else mybir.ActivationFunctionType
BF = mybir.dt.bfloat16
F32 = mybir.dt.float32
RDESTS = [(0, k) for k in range(NC)]


def build(T):
    TG = T // NC  # output steps per core
    nc = bass.Bass(target_bir_lowering=False, num_devices=NC)

    # ---- DRAM parameters (per core) ----
    inpT = nc.declare_dram_parameter("inpT", [KP, T * B], BF, isOutput=False)
    wc = nc.declare_dram_parameter("wc", [KP, GC], BF, isOutput=False)
    w0h = nc.declare_dram_parameter("w0h", [NN, GC], BF, isOutput=False)
    w1x = nc.declare_dram_parameter("w1x", [NN, GC], BF, isOutput=False)
    w1h = nc.declare_dram_parameter("w1h", [NN, GC], BF, isOutput=False)
    b1r = nc.declare_dram_parameter("b1r", [1, GC], BF, isOutput=False)
    ow0 = nc.declare_dram_parameter("ow0", [NN, NN], BF, isOutput=False)
    ob0c = nc.declare_dram_parameter("ob0c", [128, NC], F32, isOutput=False)
    ow1 = nc.declare_dram_parameter("ow1", [NN, VOCAB], BF, isOutput=False)
    ob1r = nc.declare_dram_parameter("ob1r", [1, VOCAB], BF, isOutput=False)
    iden = nc.declare_dram_parameter("iden", [128, 128], BF, isOutput=False)
    out = nc.declare_dram_parameter("out", [TG, B, VOCAB], F32, isOutput=True)

    # ---- internal DRAM ----
    # h1full[:, t*NN:(t+1)*NN] = full h1T(t): 8 unit-tiles of [128u, 128B]
    h1full = nc.dram_tensor("h1full", [128, T * NN], BF)

    # ---- SBUF ----
    wc_sb = nc.alloc_sbuf_tensor("wc_sb", [128, 3 * GC], BF)
    w0h_sb = nc.alloc_sbuf_tensor("w0h_sb", [128, 8 * GC], BF)
    w1x_sb = nc.alloc_sbuf_tensor("w1x_sb", [128, 8 * GC], BF)
    w1h_sb = nc.alloc_sbuf_tensor("w1h_sb", [128, 8 * GC], BF)
    b1_sb = nc.alloc_sbuf_tensor("b1_sb", [1, GC], BF)
    ones_sb = nc.alloc_sbuf_tensor("ones_sb", [1, 128], BF)
    id_sb = nc.alloc_sbuf_tensor("id_sb", [128, 128], BF)
    xb_sb = nc.alloc_sbuf_tensor("xb_sb", [128, 4 * 3 * 128], BF)  # 4 bufs x 3 tiles
    # gather rings: slot s, sender j tile at [:, s*NN + j*128]
    gath0 = nc.alloc_sbuf_tensor("gath0", [128, NSLOT * NN], BF)
    gath1 = nc.alloc_sbuf_tensor("gath1", [128, NSLOT * NN], BF)
    g0_sb = nc.alloc_sbuf_tensor("g0_sb", [128, 3 * SL], F32)
    t0_sb = nc.alloc_sbuf_tensor("t0_sb", [128, SL], F32)
    th0_sb = nc.alloc_sbuf_tensor("th0_sb", [128, SL], F32)
    g1_sb = nc.alloc_sbuf_tensor("g1_sb", [128, 3 * SL], F32)
    t1_sb = nc.alloc_sbuf_tensor("t1_sb", [128, SL], F32)
    th1_sb = nc.alloc_sbuf_tensor("th1_sb", [128, SL], F32)
    c0_sb = nc.alloc_sbuf_tensor("c0_sb", [128, SL], F32)
    c1_sb = nc.alloc_sbuf_tensor("c1_sb", [128, SL], F32)
    tmpa_sb = nc.alloc_sbuf_tensor("tmpa_sb", [128, SL], F32)
    tmpb_sb = nc.alloc_sbuf_tensor("tmpb_sb", [128, SL], F32)
    h0l_sb = nc.alloc_sbuf_tensor("h0l_sb", [128, SL], BF)
    h1l_sb = nc.alloc_sbuf_tensor("h1l_sb", [128, SL], BF)
    send0 = nc.alloc_sbuf_tensor("send0", [128, 2 * SL], BF)  # 2 slots
    send1 = nc.alloc_sbuf_tensor("send1", [128, 2 * SL], BF)  # 2 slots
    # output stage
    ow0_sb = nc.alloc_sbuf_tensor("ow0_sb", [128, 8 * NN], BF)
    ow1_sb = nc.alloc_sbuf_tensor("ow1_sb", [128, 8 * VOCAB], BF)
    ob0_sb = nc.alloc_sbuf_tensor("ob0_sb", [128, NC], F32)
    ob1_sb = nc.alloc_sbuf_tensor("ob1_sb", [1, VOCAB], BF)
    h14_sb = nc.alloc_sbuf_tensor("h14_sb", [128, 2 * 8 * 512], BF)  # 2 bufs
    hsT_sb = nc.alloc_sbuf_tensor("hsT_sb", [128, 2 * 8 * 512], BF)  # 2 bufs
    lg_sb = nc.alloc_sbuf_tensor("lg_sb", [128, 4 * VOCAB], F32)

    # ---- PSUM (8 banks total) ----
    psA = nc.alloc_psum_tensor("psA", [128, 512], F32)
    psB = nc.alloc_psum_tensor("psB", [128, 512], F32)
    psT0 = nc.alloc_psum_tensor("psT0", [128, 128], BF)
    psT1 = nc.alloc_psum_tensor("psT1", [128, 128], BF)
    hs_ps = [nc.alloc_psum_tensor(f"hs_ps{i}", [128, 512], F32) for i in range(4)]

    # ---- semaphores ----
    sems = {}
    for name in (
        "s_x0", "s_x1", "s_x2", "s_x3",
        "s_init", "s_vinit", "s_xdone", "s_z0", "s_z1", "s_act0",
        "s_act1", "s_cmid0", "s_cmid1", "s_th0", "s_th1", "s_dve0", "s_dve1",
        "s_t0", "s_t1", "s_cp0", "s_cp1",
        "s_prep0", "s_prep1", "rsem0", "rsem1", "lsem0", "lsem1", "s_h1st",
        "s_og", "s_ohsT", "s_orelu", "s_olog", "s_ocp", "s_out",
    ):
        sems[name] = nc.alloc_semaphore(name)

    NGR = TG // 4  # output groups of 4 steps
    NP = T + 2     # periods 0..T+1

    # bc0 trigger k (=h0(k)) fires in period k,   k = 0..T-1
    # bc1 trigger k fires in period 0 (k=0, zeros=h1(-1)) or k+1 (k>=1,
    #   h1(k-1)), k = 0..T
    # gath0 slot k%NSLOT;  gath1 slot k%NSLOT.
    # rsem0 >= 16*(k+1) <=> bc0 k arrived;  same for rsem1/bc1.

    with nc.Block() as block:

        @block.sync
        def _(sync):
            n_init = 0

            def ld(dst, src):
                nonlocal n_init
                sync.dma_start(out=dst, in_=src).then_inc(sems["s_init"], 16)
                n_init += 1

            for j in range(3):
                ld(wc_sb[:, j * GC:(j + 1) * GC], wc[j * 128:(j + 1) * 128, :])
            for j in range(8):
                ld(w0h_sb[:, j * GC:(j + 1) * GC], w0h[j * 128:(j + 1) * 128, :])
                ld(w1x_sb[:, j * GC:(j + 1) * GC], w1x[j * 128:(j + 1) * 128, :])
                ld(w1h_sb[:, j * GC:(j + 1) * GC], w1h[j * 128:(j + 1) * 128, :])
                ld(ow0_sb[:, j * NN:(j + 1) * NN], ow0[j * 128:(j + 1) * 128, :])
                ld(ow1_sb[:, j * VOCAB:(j + 1) * VOCAB], ow1[j * 128:(j + 1) * 128, :])
            ld(b1_sb[:, :], b1r[:, :])
            ld(id_sb[:, :], iden[:, :])
            ld(ob0_sb[:, :], ob0c[:, :])
            ld(ob1_sb[:, :], ob1r[:, :])
            assert n_init == 47, n_init
            inpT3 = inpT.rearrange("(j p) n -> p j n", p=128)

            def ldx(t):
                # x block t -> xb slot t%4; per-slot sem (race-free)
                if t >= 4:
                    sync.wait_ge(sems["s_xdone"], t - 3)
                s = (t % 4) * 3 * 128
                dst = xb_sb[:, s:s + 3 * 128].rearrange("p (j c) -> p j c", c=128)
                sync.dma_start(
                    out=dst, in_=inpT3[:, :, t * B:(t + 1) * B]
                ).then_inc(sems[f"s_x{t % 4}"], 16)

            # interleave x prefetch (2 periods ahead) and h1full stores
            ldx(0)
            if T > 1:
                ldx(1)
            for p in range(NP):
                if p + 2 <= T - 1:
                    ldx(p + 2)
                # store h1(tau), tau = p-2, from gath1 slot (tau+1)%NSLOT
                tau = p - 2
                if 0 <= tau <= T - 1:
                    k1 = tau + 1
                    sync.wait_ge(sems["rsem1"], 16 * (k1 + 1))
                    sync.dma_start(
                        out=h1full[:, tau * NN:(tau + 1) * NN],
                        in_=gath1[:, (k1 % NSLOT) * NN:((k1 % NSLOT) + 1) * NN],
                    ).then_inc(sems["s_h1st"], 16)

        @block.tensor
        def _(tensor):
            tensor.wait_ge(sems["s_init"], 16 * 47)
            tensor.wait_ge(sems["s_vinit"], 1)
            for p in range(NP):
                t = p
                m = p - 1
                if t <= T - 1:
                    # ---- z0x(t) into psA ----
                    tensor.wait_ge(sems[f"s_x{t % 4}"], 16 * (t // 4 + 1))
                    if t >= 1:
                        tensor.wait_ge(sems["s_act0"], t)  # psA WAR
                    xoff = (t % 4) * 3 * 128
                    for j in range(3):
                        ins = tensor.matmul(
                            psA[:, :],
                            xb_sb[:, xoff + j * 128:xoff + (j + 1) * 128],
                            wc_sb[:, j * GC:(j + 1) * GC],
                            start=(j == 0),
                            stop=(t == 0 and j == 2),
                        )
                        if j == 2:
                            ins.then_inc(sems["s_xdone"], 1)
                    # ---- z0h(t) ----
                    if t >= 1:
                        tensor.wait_ge(sems["rsem0"], 16 * t)  # h0(t-1) arrived
                        g0off = ((t - 1) % NSLOT) * NN
                        for j in range(8):
                            ins = tensor.matmul(
                                psA[:, :],
                                gath0[:, g0off + j * 128:g0off + (j + 1) * 128],
                                w0h_sb[:, j * GC:(j + 1) * GC],
                                start=False,
                                stop=(j == 7),
                            )
                            if j == 7:
                                ins.then_inc(sems["s_z0"], 1)
                # ---- TP1(p-2): transpose h1l(p-2) ----
                tau = p - 2
                if 0 <= tau <= T - 1:
                    tensor.wait_ge(sems["s_dve1"], tau + 1)
                    if tau >= 1:
                        tensor.wait_ge(sems["s_cp1"], tau)  # psT1 WAR
                    tensor.transpose(psT1[:, 0:128], h1l_sb[:, :], id_sb[:, :]).then_inc(
                        sems["s_t1"], 1
                    )
                if 1 <= p <= T:
                    # ---- z1(m): bias + h0-part into psB ----
                    if m >= 1:
                        tensor.wait_ge(sems["s_act1"], m)  # psB WAR
                    tensor.matmul(psB[:, :], ones_sb[:, :], b1_sb[:, :], start=True, stop=False)
                    tensor.wait_ge(sems["rsem0"], 16 * (m + 1))  # h0(m) arrived
                    g0off = ((t - 1) % NSLOT) * NN  # h0(m) = bc0 k=m
                    for j in range(8):
                        tensor.matmul(
                            psB[:, :],
                            gath0[:, g0off + j * 128:g0off + (j + 1) * 128],
                            w1x_sb[:, j * GC:(j + 1) * GC],
                            start=False, stop=False,
                        )
                # ---- TP0(t): transpose h0l(t) ----
                if t <= T - 1:
                    tensor.wait_ge(sems["s_dve0"], t + 1)
                    if t >= 1:
                        tensor.wait_ge(sems["s_cp0"], t)  # psT0 WAR
                    tensor.transpose(psT0[:, 0:128], h0l_sb[:, :], id_sb[:, :]).then_inc(
                        sems["s_t0"], 1
                    )
                if 1 <= p <= T:
                    # ---- z1(m): h1-part ----  h1(m-1) = bc1 k=m
                    tensor.wait_ge(sems["rsem1"], 16 * (m + 1))
                    g1off = (m % NSLOT) * NN
                    for j in range(8):
                        ins = tensor.matmul(
                            psB[:, :],
                            gath1[:, g1off + j * 128:g1off + (j + 1) * 128],
                            w1h_sb[:, j * GC:(j + 1) * GC],
                            start=False, stop=(j == 7),
                        )
                        if j == 7:
                            ins.then_inc(sems["s_z1"], 1)
            # ---- output stage ----
            for g in range(NGR):
                tensor.wait_ge(sems["s_og"], 16 * 8 * (g + 1))
                hb = (g % 2) * 8 * 512
                if g >= 1:
                    tensor.wait_ge(sems["s_orelu"], 8 * g)
                    tensor.wait_ge(sems["s_ocp"], 4 * g)
                for mm in range(4):
                    for j in range(8):
                        ins = tensor.matmul(
                            hs_ps[mm][:, :],
                            ow0_sb[:, j * NN + mm * 128:j * NN + (mm + 1) * 128],
                            h14_sb[:, hb + j * 512:hb + (j + 1) * 512],
                            start=(j == 0), stop=(j == 7),
                        )
                        if j == 7:
                            ins.then_inc(sems["s_ohsT"], 1)
                tensor.wait_ge(sems["s_orelu"], 8 * g + 4)
                for mm in range(4, 8):
                    for j in range(8):
                        ins = tensor.matmul(
                            hs_ps[mm - 4][:, :],
                            ow0_sb[:, j * NN + mm * 128:j * NN + (mm + 1) * 128],
                            h14_sb[:, hb + j * 512:hb + (j + 1) * 512],
                            start=(j == 0), stop=(j == 7),
                        )
                        if j == 7:
                            ins.then_inc(sems["s_ohsT"], 1)
                tensor.wait_ge(sems["s_orelu"], 8 * (g + 1))
                ps_l = [psA, psB, hs_ps[0], hs_ps[1]]
                sb = (g % 2) * 8 * 512
                for tau in range(4):
                    tensor.matmul(
                        ps_l[tau][:, 0:VOCAB], ones_sb[:, :], ob1_sb[:, :],
                        start=True, stop=False,
                    )
                    for mm in range(8):
                        ins = tensor.matmul(
                            ps_l[tau][:, 0:VOCAB],
                            hsT_sb[:, sb + mm * 512 + tau * 128:sb + mm * 512 + (tau + 1) * 128],
                            ow1_sb[:, mm * VOCAB:(mm + 1) * VOCAB],
                            start=False, stop=(mm == 7),
                        )
                        if mm == 7:
                            ins.then_inc(sems["s_olog"], 1)

        @block.scalar
        def _(scalar):
            scalar.wait_ge(sems["s_init"], 16 * 47)
            SIG = AF.Sigmoid
            TANH = AF.Tanh
            for p in range(NP):
                t = p
                m = p - 1
                if t <= T - 1:
                    if t == 0:
                        scalar.wait_ge(sems["s_xdone"], 1)
                    else:
                        scalar.wait_ge(sems["s_z0"], t)
                    if t >= 1:
                        scalar.wait_ge(sems["s_cmid0"], t)   # g0 f/i WAR
                        scalar.wait_ge(sems["s_dve0"], t)    # g0 o-part WAR
                    scalar.activation(g0_sb[:, :], psA[:, 0:3 * SL], SIG)
                    scalar.activation(t0_sb[:, :], psA[:, 3 * SL:4 * SL], TANH).then_inc(
                        sems["s_act0"], 1
                    )
                    scalar.wait_ge(sems["s_cmid0"], t + 1)
                    scalar.activation(th0_sb[:, :], c0_sb[:, :], TANH).then_inc(
                        sems["s_th0"], 1
                    )
                if 1 <= p <= T:
                    scalar.wait_ge(sems["s_z1"], m + 1)
                    if m >= 1:
                        scalar.wait_ge(sems["s_cmid1"], m)
                        scalar.wait_ge(sems["s_dve1"], m)
                    scalar.activation(g1_sb[:, :], psB[:, 0:3 * SL], SIG)
                    scalar.activation(t1_sb[:, :], psB[:, 3 * SL:4 * SL], TANH).then_inc(
                        sems["s_act1"], 1
                    )
                    scalar.wait_ge(sems["s_cmid1"], m + 1)
                    scalar.activation(th1_sb[:, :], c1_sb[:, :], TANH).then_inc(
                        sems["s_th1"], 1
                    )
            # output: relu with per-partition bias
            for g in range(NGR):
                sb = (g % 2) * 8 * 512
                for mm in range(8):
                    scalar.wait_ge(sems["s_ohsT"], 8 * g + mm + 1)
                    if g >= 2:
                        scalar.wait_ge(sems["s_olog"], 4 * (g - 1))
                    scalar.activation(
                        hsT_sb[:, sb + mm * 512:sb + (mm + 1) * 512],
                        hs_ps[mm % 4][:, :],
                        AF.Relu,
                        bias=ob0_sb[:, mm:mm + 1],
                    ).then_inc(sems["s_orelu"], 1)

        @block.vector
        def _(vector):
            vector.memset(send0[:, :], 0.0)
            vector.memset(send1[:, :], 0.0)
            vector.memset(c0_sb[:, :], 0.0)
            vector.memset(c1_sb[:, :], 0.0)
            vector.memset(ones_sb[:, :], 1.0).then_inc(sems["s_vinit"], 1)
            MUL = mybir.AluOpType.mult
            for p in range(NP):
                t = p
                m = p - 1
                if t <= T - 1:
                    # cell0(t)
                    vector.wait_ge(sems["s_act0"], t + 1)
                    vector.tensor_tensor(tmpa_sb[:, :], g0_sb[:, 0:SL], c0_sb[:, :], MUL)
                    vector.tensor_tensor(tmpb_sb[:, :], g0_sb[:, SL:2 * SL], t0_sb[:, :], MUL)
                    vector.tensor_add(c0_sb[:, :], tmpa_sb[:, :], tmpb_sb[:, :]).then_inc(
                        sems["s_cmid0"], 1
                    )
                    vector.wait_ge(sems["s_th0"], t + 1)
                    vector.tensor_tensor(
                        h0l_sb[:, :], g0_sb[:, 2 * SL:3 * SL], th0_sb[:, :], MUL
                    ).then_inc(sems["s_dve0"], 1)
                    # cp0(t): psT0 -> send0 slot t%2   (bc0 k=t)
                    vector.wait_ge(sems["s_t0"], t + 1)
                    if t >= 2:
                        vector.wait_ge(sems["lsem0"], 16 * (t - 1))  # send0 WAR
                    vector.tensor_copy(
                        send0[:, (t % 2) * SL:(t % 2 + 1) * SL], psT0[:, 0:128]
                    ).then_inc(sems["s_cp0"], 1)
                # cp1(tau) FIRST: psT1(tau) -> send1 slot k1%2, k1 = tau+1.
                # Must precede cell1(m): the bc1 trigger consuming cp1 gates
                # z1(m) -> act1(m) -> cell1(m) transitively (deadlock else).
                tau = p - 2
                if 0 <= tau <= T - 1:
                    k1 = tau + 1
                    vector.wait_ge(sems["s_t1"], tau + 1)
                    if k1 >= 2:
                        vector.wait_ge(sems["lsem1"], 16 * (k1 - 1))  # send1 WAR
                    vector.tensor_copy(
                        send1[:, (k1 % 2) * SL:(k1 % 2 + 1) * SL], psT1[:, 0:128]
                    ).then_inc(sems["s_cp1"], 1)
                if 1 <= p <= T:
                    # cell1(m)
                    vector.wait_ge(sems["s_act1"], m + 1)
                    vector.tensor_tensor(tmpa_sb[:, :], g1_sb[:, 0:SL], c1_sb[:, :], MUL)
                    vector.tensor_tensor(tmpb_sb[:, :], g1_sb[:, SL:2 * SL], t1_sb[:, :], MUL)
                    vector.tensor_add(c1_sb[:, :], tmpa_sb[:, :], tmpb_sb[:, :]).then_inc(
                        sems["s_cmid1"], 1
                    )
                    vector.wait_ge(sems["s_th1"], m + 1)
                    vector.tensor_tensor(
                        h1l_sb[:, :], g1_sb[:, 2 * SL:3 * SL], th1_sb[:, :], MUL
                    ).then_inc(sems["s_dve1"], 1)
            # output: copy logits psum -> sbuf
            ps_l = [psA, psB, hs_ps[0], hs_ps[1]]
            for g in range(NGR):
                for tau in range(4):
                    vector.wait_ge(sems["s_olog"], 4 * g + tau + 1)
                    if g >= 1:
                        vector.wait_ge(sems["s_out"], 16 * (4 * (g - 1) + tau + 1))
                    vector.tensor_copy(
                        lg_sb[:, tau * VOCAB:(tau + 1) * VOCAB], ps_l[tau][:, 0:VOCAB]
                    ).then_inc(sems["s_ocp"], 1)

        @block.gpsimd
        def _(gpsimd):
            gpsimd.load_library(library_config.remote_dma)
            rank = gpsimd.partition_id()
            rreg = gpsimd.to_reg(rank)

            rank_off = rank * 128  # out slot offset for own tile (elements)

            def prep_both(k1n, k0n):
                # register-offset out slot: no branching on the hot path
                if k1n is not None:
                    base1 = (k1n % NSLOT) * NN
                    gpsimd.remote_dma_broadcast(
                        gath1[:, bass.ds(base1 + rank_off, 128)],
                        send1[:, (k1n % 2) * SL:(k1n % 2 + 1) * SL],
                        sems["rsem1"], sems["lsem1"],
                        rdests=RDESTS,
                    ).then_inc(sems["s_prep1"], 1)
                if k0n is not None:
                    base0 = (k0n % NSLOT) * NN
                    gpsimd.remote_dma_broadcast(
                        gath0[:, bass.ds(base0 + rank_off, 128)],
                        send0[:, (k0n % 2) * SL:(k0n % 2 + 1) * SL],
                        sems["rsem0"], sems["lsem0"],
                        rdests=RDESTS,
                    ).then_inc(sems["s_prep0"], 1)

            # preps for period 0: bc1 k=0 (zeros) and bc0 k=0
            prep_both(0, 0)
            n_prep0 = 1
            n_prep1 = 1
            n_trig0 = 0
            n_trig1 = 0
            for p in range(NP):
                t = p
                # ---- trigger bc1 ----
                k1 = None
                if p == 0:
                    k1 = 0      # zeros = h1(-1); send1 slot 0 memset
                    gpsimd.wait_ge(sems["s_vinit"], 1)
                elif 2 <= p <= T + 1:
                    k1 = p - 1  # h1(p-2), copied by cp1(tau=p-2) this period
                if k1 is not None:
                    gpsimd.wait_ge(sems["s_prep1"], n_trig1 + 1)
                    if k1 >= 1:
                        gpsimd.wait_ge(sems["s_cp1"], k1)  # cp1(tau=k1-1) done
                    if k1 >= 3:
                        # gath1 slot WAR vs own h1full store (transitive)
                        gpsimd.wait_ge(sems["s_h1st"], 16 * (k1 - 2))
                    gpsimd.trigger_dma(1)
                    n_trig1 += 1
                # ---- trigger bc0 k=t ----
                if t <= T - 1:
                    gpsimd.wait_ge(sems["s_prep0"], n_trig0 + 1)
                    gpsimd.wait_ge(sems["s_cp0"], t + 1)
                    gpsimd.trigger_dma(1)
                    n_trig0 += 1
                # ---- preps for next period ----
                np_ = p + 1
                k1n = np_ - 1 if 2 <= np_ <= T + 1 else None
                k0n = np_ if np_ <= T - 1 else None
                if k1n is not None or k0n is not None:
                    prep_both(k1n, k0n)
                    n_prep1 += k1n is not None
                    n_prep0 += k0n is not None
            assert n_trig0 == T and n_trig1 == T + 1, (n_trig0, n_trig1)
            assert n_prep0 == T and n_prep1 == T + 1, (n_prep0, n_prep1)

            # ---- output stage: loads + stores ----
            TG = T // NC
            NGR_ = TG // 4
            for g in range(NGR_):
                if g >= 2:
                    gpsimd.wait_ge(sems["s_ohsT"], 8 * (g - 1))
                hb = (g % 2) * 8 * 512
                if g == 0:
                    # all T h1full stores done (race-free total; the loop has
                    # just ended for every core anyway)
                    gpsimd.wait_ge(sems["s_h1st"], 16 * T)
                for k in range(NC):
                    with gpsimd.If_eq(rreg, k):
                        t0g = k * TG + g * 4
                        h1f3 = h1full.rearrange("p (s c) -> p s c", c=NN)
                        for j in range(8):
                            # 3D AP: 4 steps (stride NN) x 128 cols
                            gpsimd.dma_start(
                                out=h14_sb[:, hb + j * 512:hb + (j + 1) * 512]
                                .rearrange("p (s c) -> p s c", c=128),
                                in_=h1f3[:, t0g:t0g + 4, j * 128:(j + 1) * 128],
                            ).then_inc(sems["s_og"], 16)
                if g >= 1:
                    for tau in range(4):
                        gpsimd.wait_ge(sems["s_ocp"], 4 * (g - 1) + tau + 1)
                        gpsimd.dma_start(
                            out=out[4 * (g - 1) + tau, :, :],
                            in_=lg_sb[:, tau * VOCAB:(tau + 1) * VOCAB],
                        ).then_inc(sems["s_out"], 16)
            g = NGR_
            for tau in range(4):
                gpsimd.wait_ge(sems["s_ocp"], 4 * (g - 1) + tau + 1)
                gpsimd.dma_start(
                    out=out[4 * (g - 1) + tau, :, :],
                    in_=lg_sb[:, tau * VOCAB:(tau + 1) * VOCAB],
                ).then_inc(sems["s_out"], 16)

    return nc


def _host_prep(inputs, T):
    inp = np.ascontiguousarray(inputs["inputs"][:T]).astype(np.float32)
    emb_W = inputs["emb_W"].astype(np.float32)
    W0 = inputs["lstm_W0"].astype(np.float32)
    b0 = inputs["lstm_b0"].astype(np.float32)
    W1 = inputs["lstm_W1"].astype(np.float32)
    b1 = inputs["lstm_b1"].astype(np.float32)

    flat = inp.reshape(T * B, IND)
    s = np.where(
        (flat[:, VOCAB] == 1.0) & (flat[:, VOCAB + 1] == 0.0), 1.0, -1.0
    ).astype(np.float32)
    inpT_aug = np.zeros((KP, T * B), np.float32)
    inpT_aug[:IND] = flat.T
    inpT_aug[IND] = 1.0
    inpT_aug[IND + 1] = s

    # x-side folded weight: emb @ W0[:512] + flags(b0 row) + rank1(u row)
    Wc = np.zeros((KP, 4 * NN), np.float32)
    Wc[:IND] = emb_W @ W0[:EMB]
    Wc[IND] = b0
    Wc[IND + 1] = W0[EMB:EMB + BIG].sum(axis=0)

    W0h = W0[EMB + BIG:]            # [1024, 4096]
    W1x, W1h = W1[:NN], W1[NN:]

    def gate_cols(W, k):
        return np.concatenate(
            [W[:, base + k * SL:base + (k + 1) * SL] for base in
             (0, NN, 2 * NN, 3 * NN)], axis=1)

    bf = lambda x: np.ascontiguousarray(x).astype(BF16)
    inpT_bf = bf(inpT_aug)
    ow0 = bf(inputs["out_W0"])
    ob0c = np.ascontiguousarray(
        inputs["out_b0"].astype(np.float32).reshape(NC, 128).T
    )
    ow1 = bf(inputs["out_W1"])
    ob1r = bf(inputs["out_b1"].reshape(1, VOCAB))
    iden = bf(np.eye(128, dtype=np.float32))

    in_maps = []
    for k in range(NC):
        in_maps.append({
            "inpT": inpT_bf,
            "wc": bf(gate_cols(Wc, k)),
            "w0h": bf(gate_cols(W0h, k)),
            "w1x": bf(gate_cols(W1x, k)),
            "w1h": bf(gate_cols(W1h, k)),
            "b1r": bf(gate_cols(b1.reshape(1, 4 * NN), k)),
            "ow0": ow0,
            "ob0c": ob0c,
            "ow1": ow1,
            "ob1r": ob1r,
            "iden": iden,
        })
    return in_maps


_CACHE = {}


def run(inputs, T=T_FULL, trace=False):
    if T not in _CACHE:
        nc_new = build(T)
        lower_extended_insts(nc_new)
        _CACHE[T] = nc_new
    nc = _CACHE[T]
    in_maps = _host_prep(inputs, T)
    res = run_bass_kernel_spmd(
        nc, in_maps, core_ids=list(range(NC)), trace=trace
    )
    out = np.concatenate([res.results[k]["out"] for k in range(NC)], axis=0)
    return out, res


def kernel(**inputs):
    out, _ = run(inputs, T=T_FULL)
    return out.astype(np.float32)


# revision 6
# speedup vs baseline: 1.1189x; 1.1189x over previous
"""Trainium2 Bass kernel for the 2-layer LSTM language model — v3.

8-way tensor parallelism over the hidden/gate dimension; per-step hidden
exchange via direct SBUF->SBUF remote DMA broadcasts (SWDGE
remote_dma_broadcast, register-offset out slot), split per layer and
triggered as soon as each h tile is transposed.

Per period p (t=p, m=p-1, j=p-2) the tensor engine runs:
    z0x(t) | z0h(t) | z1h1(m-1) | TP0(t) | b1+z1h0(m) | TP1(j)
i.e. z1's h1-part runs at the FRONT of the next period, so the bc1
broadcast it consumes (h1 two steps back) has a full period of flight
slack, and bc0/bc1 trigger mid-period right after their transposes.
z1's PSUM accumulation spans two periods on ping-ponged psB banks.

Every core receives every h1T tile each step; the gathered [128,1024]
slab goes to private DRAM (h1full) so the output MLP (unchanged from v1:
transposed hs, per-partition relu bias, 4-step groups) needs no final
collective.  Embedding + speaker-flag rank-1 term + b0 folded into a
single [384,4096] input-side weight on the host.  Matmuls bf16 with f32
PSUM; cell state f32.
"""
import numpy as np
import ml_dtypes

import concourse.bass as bass
import concourse.mybir as mybir
from concourse import library_config
from concourse.bass_utils import run_bass_kernel_spmd
from concourse.library_overlay import lower_extended_insts

BF16 = ml_dtypes.bfloat16

T_FULL, B, IND = 512, 128, 259
EMB, NN, VOCAB, BIG = 512, 1024, 256, 128
NC = 8
SL = NN // NC          # 128 hidden units per core
GC = 4 * SL            # 512 gate columns per core
KP = 384               # padded inpT rows = 3 K-tiles (259 data + 1 + s + pad)
NSLOT = 4              # gather ring slots
AF = mybir.dt.ActivationFunctionType if hasattr(mybir.dt, "ActivationFunctionType") else mybir.ActivationFunctionType
BF = mybir.dt.bfloat16
F32 = mybir.dt.float32
RDESTS = [(0, k) for k in range(NC)]


def build(T):
    TG = T // NC  # output steps per core
    nc = bass.Bass(target_bir_lowering=False, num_devices=NC)

    # ---- DRAM parameters (per core) ----
    inpT = nc.declare_dram_parameter("inpT", [KP, T * B], BF, isOutput=False)
    wc = nc.declare_dram_parameter("wc", [KP, GC], BF, isOutput=False)
    w0h = nc.declare_dram_parameter("w0h", [NN, GC], BF, isOutput=False)
    w1x = nc.declare_dram_parameter("w1x", [NN, GC], BF, isOutput=False)
    w1h = nc.declare_dram_parameter("w1h", [NN, GC], BF, isOutput=False)
    b1r = nc.declare_dram_parameter("b1r", [1, GC], BF, isOutput=False)
    ow0 = nc.declare_dram_parameter("ow0", [NN, NN], BF, isOutput=False)
    ob0c = nc.declare_dram_parameter("ob0c", [128, NC], F32, isOutput=False)
    ow1 = nc.declare_dram_parameter("ow1", [NN, VOCAB], BF, isOutput=False)
    ob1r = nc.declare_dram_parameter("ob1r", [1, VOCAB], BF, isOutput=False)
    iden = nc.declare_dram_parameter("iden", [128, 128], BF, isOutput=False)
    out = nc.declare_dram_parameter("out", [TG, B, VOCAB], F32, isOutput=True)

    # ---- internal DRAM ----
    # h1full[:, t*NN:(t+1)*NN] = full h1T(t): 8 unit-tiles of [128u, 128B]
    h1full = nc.dram_tensor("h1full", [128, T * NN], BF)

    # ---- SBUF ----
    wc_sb = nc.alloc_sbuf_tensor("wc_sb", [128, 3 * GC], BF)
    w0h_sb = nc.alloc_sbuf_tensor("w0h_sb", [128, 8 * GC], BF)
    w1x_sb = nc.alloc_sbuf_tensor("w1x_sb", [128, 8 * GC], BF)
    w1h_sb = nc.alloc_sbuf_tensor("w1h_sb", [128, 8 * GC], BF)
    b1_sb = nc.alloc_sbuf_tensor("b1_sb", [1, GC], BF)
    ones_sb = nc.alloc_sbuf_tensor("ones_sb", [1, 128], BF)
    id_sb = nc.alloc_sbuf_tensor("id_sb", [128, 128], BF)
    xb_sb = nc.alloc_sbuf_tensor("xb_sb", [128, 4 * 3 * 128], BF)  # 4 bufs x 3 tiles
    # gather rings: slot s, sender j tile at [:, s*NN + j*128]
    gath0 = nc.alloc_sbuf_tensor("gath0", [128, NSLOT * NN], BF)
    gath1 = nc.alloc_sbuf_tensor("gath1", [128, NSLOT * NN], BF)
    g0_sb = nc.alloc_sbuf_tensor("g0_sb", [128, 3 * SL], F32)
    t0_sb = nc.alloc_sbuf_tensor("t0_sb", [128, SL], F32)
    th0_sb = nc.alloc_sbuf_tensor("th0_sb", [128, SL], F32)
    g1_sb = nc.alloc_sbuf_tensor("g1_sb", [128, 3 * SL], F32)
    t1_sb = nc.alloc_sbuf_tensor("t1_sb", [128, SL], F32)
    th1_sb = nc.alloc_sbuf_tensor("th1_sb", [128, SL], F32)
    c0_sb = nc.alloc_sbuf_tensor("c0_sb", [128, SL], F32)
    c1_sb = nc.alloc_sbuf_tensor("c1_sb", [128, SL], F32)
    tmpa_sb = nc.alloc_sbuf_tensor("tmpa_sb", [128, SL], F32)
    tmpb_sb = nc.alloc_sbuf_tensor("tmpb_sb", [128, SL], F32)
    h0l_sb = nc.alloc_sbuf_tensor("h0l_sb", [128, SL], BF)
    h1l_sb = nc.alloc_sbuf_tensor("h1l_sb", [128, SL], BF)
    send0 = nc.alloc_sbuf_tensor("send0", [128, 2 * SL], BF)  # 2 slots
    send1 = nc.alloc_sbuf_tensor("send1", [128, 2 * SL], BF)  # 2 slots
    # output stage
    ow0_sb = nc.alloc_sbuf_tensor("ow0_sb", [128, 8 * NN], BF)
    ow1_sb = nc.alloc_sbuf_tensor("ow1_sb", [128, 8 * VOCAB], BF)
    ob0_sb = nc.alloc_sbuf_tensor("ob0_sb", [128, NC], F32)
    ob1_sb = nc.alloc_sbuf_tensor("ob1_sb", [1, VOCAB], BF)
    h14_sb = nc.alloc_sbuf_tensor("h14_sb", [128, 2 * 8 * 512], BF)  # 2 bufs
    hsT_sb = nc.alloc_sbuf_tensor("hsT_sb", [128, 2 * 8 * 512], BF)  # 2 bufs
    lg_sb = nc.alloc_sbuf_tensor("lg_sb", [128, 4 * VOCAB], F32)

    # ---- PSUM (8 banks total) ----
    psA = nc.alloc_psum_tensor("psA", [128, 512], F32)
    psB = [nc.alloc_psum_tensor(f"psB{i}", [128, 512], F32) for i in range(2)]
    psT = nc.alloc_psum_tensor("psT", [128, 256], BF)  # [0:128]=TP0 [128:256]=TP1
    hs_ps = [nc.alloc_psum_tensor(f"hs_ps{i}", [128, 512], F32) for i in range(4)]

    # ---- semaphores ----
    sems = {}
    for name in (
        "s_x0", "s_x1", "s_x2", "s_x3",
        "s_init", "s_vinit", "s_xdone", "s_z0", "s_z1", "s_act0",
        "s_act1", "s_cmid0", "s_cmid1", "s_th0", "s_th1", "s_dve0", "s_dve1",
        "s_t0", "s_t1", "s_cp0", "s_cp1",
        "s_prep0", "s_prep1", "rsem0", "rsem1", "lsem0", "lsem1", "s_h1st",
        "s_og", "s_ohsT", "s_orelu", "s_olog", "s_ocp", "s_out",
    ):
        sems[name] = nc.alloc_semaphore(name)

    NGR = TG // 4  # output groups of 4 steps
    NP = T + 2     # periods 0..T+1

    # bc0 k carries h0(k): cp0(k)/trigger in period k, k = 0..T-1.
    # bc1 k carries h1(k-1): k=0 zeros (trigger period 0), k>=1 trigger in
    #   period k+1 after cp1(j=k-1).  k = 0..T.
    # gath slots k%NSLOT; send slots k%2.
    # rsem >= 16*(k+1) <=> bc k fully arrived.
    # consumers: z0h(t)+z1h0(t-1) read bc0 k=t-1; z1h1(m-1) in period p
    #   reads bc1 k=p-2.

    with nc.Block() as block:

        @block.sync
        def _(sync):
            n_init = 0

            def ld(dst, src):
                nonlocal n_init
                sync.dma_start(out=dst, in_=src).then_inc(sems["s_init"], 16)
                n_init += 1

            for j in range(3):
                ld(wc_sb[:, j * GC:(j + 1) * GC], wc[j * 128:(j + 1) * 128, :])
            for j in range(8):
                ld(w0h_sb[:, j * GC:(j + 1) * GC], w0h[j * 128:(j + 1) * 128, :])
                ld(w1x_sb[:, j * GC:(j + 1) * GC], w1x[j * 128:(j + 1) * 128, :])
                ld(w1h_sb[:, j * GC:(j + 1) * GC], w1h[j * 128:(j + 1) * 128, :])
                ld(ow0_sb[:, j * NN:(j + 1) * NN], ow0[j * 128:(j + 1) * 128, :])
                ld(ow1_sb[:, j * VOCAB:(j + 1) * VOCAB], ow1[j * 128:(j + 1) * 128, :])
            ld(b1_sb[:, :], b1r[:, :])
            ld(id_sb[:, :], iden[:, :])
            ld(ob0_sb[:, :], ob0c[:, :])
            ld(ob1_sb[:, :], ob1r[:, :])
            assert n_init == 47, n_init
            inpT3 = inpT.rearrange("(j p) n -> p j n", p=128)

            def ldx(t):
                # x block t -> xb slot t%4; per-slot sem (race-free)
                if t >= 4:
                    sync.wait_ge(sems["s_xdone"], t - 3)
                s = (t % 4) * 3 * 128
                dst = xb_sb[:, s:s + 3 * 128].rearrange("p (j c) -> p j c", c=128)
                sync.dma_start(
                    out=dst, in_=inpT3[:, :, t * B:(t + 1) * B]
                ).then_inc(sems[f"s_x{t % 4}"], 16)

            # interleave x prefetch (2 periods ahead) and h1full stores
            ldx(0)
            if T > 1:
                ldx(1)
            for p in range(NP):
                if p + 2 <= T - 1:
                    ldx(p + 2)
                # store h1(tau), tau = p-2, from gath1 slot (tau+1)%NSLOT
                tau = p - 2
                if 0 <= tau <= T - 1:
                    k1 = tau + 1
                    sync.wait_ge(sems["rsem1"], 16 * (k1 + 1))
                    sync.dma_start(
                        out=h1full[:, tau * NN:(tau + 1) * NN],
                        in_=gath1[:, (k1 % NSLOT) * NN:((k1 % NSLOT) + 1) * NN],
                    ).then_inc(sems["s_h1st"], 16)

        @block.tensor
        def _(tensor):
            tensor.wait_ge(sems["s_init"], 16 * 47)
            tensor.wait_ge(sems["s_vinit"], 1)
            for p in range(NP):
                t = p
                m = p - 1
                j = p - 2
                if t <= T - 1:
                    # ---- B: z0x(t) into psA ----
                    tensor.wait_ge(sems[f"s_x{t % 4}"], 16 * (t // 4 + 1))
                    if t >= 1:
                        tensor.wait_ge(sems["s_act0"], t)  # psA WAR
                    xoff = (t % 4) * 3 * 128
                    for jj in range(3):
                        ins = tensor.matmul(
                            psA[:, :],
                            xb_sb[:, xoff + jj * 128:xoff + (jj + 1) * 128],
                            wc_sb[:, jj * GC:(jj + 1) * GC],
                            start=(jj == 0),
                            stop=(t == 0 and jj == 2),
                        )
                        if jj == 2:
                            ins.then_inc(sems["s_xdone"], 1)
                    # ---- C: z0h(t) ----
                    if t >= 1:
                        tensor.wait_ge(sems["rsem0"], 16 * t)  # bc0 k=t-1
                        g0off = ((t - 1) % NSLOT) * NN
                        for jj in range(8):
                            ins = tensor.matmul(
                                psA[:, :],
                                gath0[:, g0off + jj * 128:g0off + (jj + 1) * 128],
                                w0h_sb[:, jj * GC:(jj + 1) * GC],
                                start=False,
                                stop=(jj == 7),
                            )
                            if jj == 7:
                                ins.then_inc(sems["s_z0"], 1)
                if 2 <= p <= T + 1:
                    # ---- A: z1h1(m-1) closes psB[(m-1)%2] (bc1 k=p-2) ----
                    tensor.wait_ge(sems["rsem1"], 16 * (p - 1))
                    g1off = ((p - 2) % NSLOT) * NN
                    bank = psB[(m - 1) % 2]
                    for jj in range(8):
                        ins = tensor.matmul(
                            bank[:, :],
                            gath1[:, g1off + jj * 128:g1off + (jj + 1) * 128],
                            w1h_sb[:, jj * GC:(jj + 1) * GC],
                            start=False, stop=(jj == 7),
                            skip_group_check=True,
                        )
                        if jj == 7:
                            ins.then_inc(sems["s_z1"], 1)
                if t <= T - 1:
                    # ---- F: TP0(t) ----
                    tensor.wait_ge(sems["s_dve0"], t + 1)
                    if t >= 1:
                        tensor.wait_ge(sems["s_cp0"], t)  # psT[0:128] WAR
                    tensor.transpose(psT[:, 0:128], h0l_sb[:, :], id_sb[:, :]).then_inc(
                        sems["s_t0"], 1
                    )
                if 1 <= p <= T:
                    # ---- E: bias + z1h0(m) opens psB[m%2] ----
                    if m >= 2:
                        tensor.wait_ge(sems["s_act1"], m - 1)  # bank WAR
                    tensor.wait_ge(sems["rsem0"], 16 * t)  # bc0 k=m (= t-1)
                    bank = psB[m % 2]
                    tensor.matmul(bank[:, :], ones_sb[:, :], b1_sb[:, :],
                                  start=True, stop=False, skip_group_check=True)
                    g0off = ((t - 1) % NSLOT) * NN
                    for jj in range(8):
                        tensor.matmul(
                            bank[:, :],
                            gath0[:, g0off + jj * 128:g0off + (jj + 1) * 128],
                            w1x_sb[:, jj * GC:(jj + 1) * GC],
                            start=False, stop=False,
                            skip_group_check=True,
                        )
                if 2 <= p <= T + 1:
                    # ---- D: TP1(j) ----
                    tensor.wait_ge(sems["s_dve1"], j + 1)
                    if j >= 1:
                        tensor.wait_ge(sems["s_cp1"], j)  # psT[128:256] WAR
                    tensor.transpose(psT[:, 128:256], h1l_sb[:, :], id_sb[:, :]).then_inc(
                        sems["s_t1"], 1
                    )
            # ---- output stage ----
            for g in range(NGR):
                tensor.wait_ge(sems["s_og"], 16 * 8 * (g + 1))
                hb = (g % 2) * 8 * 512
                if g >= 1:
                    tensor.wait_ge(sems["s_orelu"], 8 * g)
                    tensor.wait_ge(sems["s_ocp"], 4 * g)
                for mm in range(4):
                    for jj in range(8):
                        ins = tensor.matmul(
                            hs_ps[mm][:, :],
                            ow0_sb[:, jj * NN + mm * 128:jj * NN + (mm + 1) * 128],
                            h14_sb[:, hb + jj * 512:hb + (jj + 1) * 512],
                            start=(jj == 0), stop=(jj == 7),
                        )
                        if jj == 7:
                            ins.then_inc(sems["s_ohsT"], 1)
                tensor.wait_ge(sems["s_orelu"], 8 * g + 4)
                for mm in range(4, 8):
                    for jj in range(8):
                        ins = tensor.matmul(
                            hs_ps[mm - 4][:, :],
                            ow0_sb[:, jj * NN + mm * 128:jj * NN + (mm + 1) * 128],
                            h14_sb[:, hb + jj * 512:hb + (jj + 1) * 512],
                            start=(jj == 0), stop=(jj == 7),
                        )
                        if jj == 7:
                            ins.then_inc(sems["s_ohsT"], 1)
                tensor.wait_ge(sems["s_orelu"], 8 * (g + 1))
                ps_l = [psA, psB[0], hs_ps[0], hs_ps[1]]
                sb = (g % 2) * 8 * 512
                for tau in range(4):
                    tensor.matmul(
                        ps_l[tau][:, 0:VOCAB], ones_sb[:, :], ob1_sb[:, :],
                        start=True, stop=False,
                    )
                    for mm in range(8):
                        ins = tensor.matmul(
                            ps_l[tau][:, 0:VOCAB],
                            hsT_sb[:, sb + mm * 512 + tau * 128:sb + mm * 512 + (tau + 1) * 128],
                            ow1_sb[:, mm * VOCAB:(mm + 1) * VOCAB],
                            start=False, stop=(mm == 7),
                        )
                        if mm == 7:
                            ins.then_inc(sems["s_olog"], 1)

        @block.scalar
        def _(scalar):
            scalar.wait_ge(sems["s_init"], 16 * 47)
            SIG = AF.Sigmoid
            TANH = AF.Tanh
            for p in range(NP):
                t = p
                j = p - 2
                if t <= T - 1:
                    if t == 0:
                        scalar.wait_ge(sems["s_xdone"], 1)
                    else:
                        scalar.wait_ge(sems["s_z0"], t)
                    if t >= 1:
                        scalar.wait_ge(sems["s_cmid0"], t)   # g0 f/i WAR
                        scalar.wait_ge(sems["s_dve0"], t)    # g0 o-part WAR
                    scalar.activation(g0_sb[:, :], psA[:, 0:3 * SL], SIG)
                    scalar.activation(t0_sb[:, :], psA[:, 3 * SL:4 * SL], TANH).then_inc(
                        sems["s_act0"], 1
                    )
                    scalar.wait_ge(sems["s_cmid0"], t + 1)
                    scalar.activation(th0_sb[:, :], c0_sb[:, :], TANH).then_inc(
                        sems["s_th0"], 1
                    )
                if 2 <= p <= T + 1:
                    # act1(j): z1(j) closed by A this period
                    scalar.wait_ge(sems["s_z1"], j + 1)
                    if j >= 1:
                        scalar.wait_ge(sems["s_cmid1"], j)
                        scalar.wait_ge(sems["s_dve1"], j)
                    bank = psB[j % 2]
                    scalar.activation(g1_sb[:, :], bank[:, 0:3 * SL], SIG)
                    scalar.activation(t1_sb[:, :], bank[:, 3 * SL:4 * SL], TANH).then_inc(
                        sems["s_act1"], 1
                    )
                    scalar.wait_ge(sems["s_cmid1"], j + 1)
                    scalar.activation(th1_sb[:, :], c1_sb[:, :], TANH).then_inc(
                        sems["s_th1"], 1
                    )
            # output: relu with per-partition bias
            for g in range(NGR):
                sb = (g % 2) * 8 * 512
                for mm in range(8):
                    scalar.wait_ge(sems["s_ohsT"], 8 * g + mm + 1)
                    if g >= 2:
                        scalar.wait_ge(sems["s_olog"], 4 * (g - 1))
                    scalar.activation(
                        hsT_sb[:, sb + mm * 512:sb + (mm + 1) * 512],
                        hs_ps[mm % 4][:, :],
                        AF.Relu,
                        bias=ob0_sb[:, mm:mm + 1],
                    ).then_inc(sems["s_orelu"], 1)

        @block.vector
        def _(vector):
            vector.memset(send0[:, :], 0.0)
            vector.memset(send1[:, :], 0.0)
            vector.memset(c0_sb[:, :], 0.0)
            vector.memset(c1_sb[:, :], 0.0)
            vector.memset(ones_sb[:, :], 1.0).then_inc(sems["s_vinit"], 1)
            MUL = mybir.AluOpType.mult
            for p in range(NP):
                t = p
                j = p - 2
                if t <= T - 1:
                    # cell0(t)
                    vector.wait_ge(sems["s_act0"], t + 1)
                    vector.tensor_tensor(tmpa_sb[:, :], g0_sb[:, 0:SL], c0_sb[:, :], MUL)
                    vector.tensor_tensor(tmpb_sb[:, :], g0_sb[:, SL:2 * SL], t0_sb[:, :], MUL)
                    vector.tensor_add(c0_sb[:, :], tmpa_sb[:, :], tmpb_sb[:, :]).then_inc(
                        sems["s_cmid0"], 1
                    )
                    vector.wait_ge(sems["s_th0"], t + 1)
                    vector.tensor_tensor(
                        h0l_sb[:, :], g0_sb[:, 2 * SL:3 * SL], th0_sb[:, :], MUL
                    ).then_inc(sems["s_dve0"], 1)
                    # cp0(t): psT[0:128] -> send0 slot t%2  (bc0 k=t)
                    vector.wait_ge(sems["s_t0"], t + 1)
                    if t >= 2:
                        vector.wait_ge(sems["lsem0"], 16 * (t - 1))  # send0 WAR
                    vector.tensor_copy(
                        send0[:, (t % 2) * SL:(t % 2 + 1) * SL], psT[:, 0:128]
                    ).then_inc(sems["s_cp0"], 1)
                if 2 <= p <= T + 1:
                    # cell1(j)
                    vector.wait_ge(sems["s_act1"], j + 1)
                    vector.tensor_tensor(tmpa_sb[:, :], g1_sb[:, 0:SL], c1_sb[:, :], MUL)
                    vector.tensor_tensor(tmpb_sb[:, :], g1_sb[:, SL:2 * SL], t1_sb[:, :], MUL)
                    vector.tensor_add(c1_sb[:, :], tmpa_sb[:, :], tmpb_sb[:, :]).then_inc(
                        sems["s_cmid1"], 1
                    )
                    vector.wait_ge(sems["s_th1"], j + 1)
                    vector.tensor_tensor(
                        h1l_sb[:, :], g1_sb[:, 2 * SL:3 * SL], th1_sb[:, :], MUL
                    ).then_inc(sems["s_dve1"], 1)
                    # cp1(j): psT[128:256] -> send1 slot k1%2, k1 = j+1
                    k1 = j + 1
                    vector.wait_ge(sems["s_t1"], j + 1)
                    if k1 >= 2:
                        vector.wait_ge(sems["lsem1"], 16 * (k1 - 1))  # send1 WAR
                    vector.tensor_copy(
                        send1[:, (k1 % 2) * SL:(k1 % 2 + 1) * SL], psT[:, 128:256]
                    ).then_inc(sems["s_cp1"], 1)
            # output: copy logits psum -> sbuf
            ps_l = [psA, psB[0], hs_ps[0], hs_ps[1]]
            for g in range(NGR):
                for tau in range(4):
                    vector.wait_ge(sems["s_olog"], 4 * g + tau + 1)
                    if g >= 1:
                        vector.wait_ge(sems["s_out"], 16 * (4 * (g - 1) + tau + 1))
                    vector.tensor_copy(
                        lg_sb[:, tau * VOCAB:(tau + 1) * VOCAB], ps_l[tau][:, 0:VOCAB]
                    ).then_inc(sems["s_ocp"], 1)

        @block.gpsimd
        def _(gpsimd):
            gpsimd.load_library(library_config.remote_dma)
            rank = gpsimd.partition_id()
            rreg = gpsimd.to_reg(rank)
            rank_off = rank * 128  # out slot offset for own tile (elements)

            def prep_bc(q, k):
                gd, sd = (gath0, send0) if q == 0 else (gath1, send1)
                gpsimd.remote_dma_broadcast(
                    gd[:, bass.ds((k % NSLOT) * NN + rank_off, 128)],
                    sd[:, (k % 2) * SL:(k % 2 + 1) * SL],
                    sems["rsem0" if q == 0 else "rsem1"],
                    sems["lsem0" if q == 0 else "lsem1"],
                    rdests=RDESTS,
                ).then_inc(sems["s_prep0" if q == 0 else "s_prep1"], 1)

            def trig_list(p):
                lst = []
                if p == 0:
                    lst.append((1, 0))     # bc1 k=0: zeros = h1(-1)
                if p <= T - 1:
                    lst.append((0, p))     # bc0 k=p
                if 2 <= p <= T + 1:
                    lst.append((1, p - 1))  # bc1 k=p-1 carries h1(p-2)
                return lst

            # preps for period 0 (FIFO order = trigger order)
            for q, k in trig_list(0):
                prep_bc(q, k)
            n_trig = [0, 0]
            for p in range(NP):
                for q, k in trig_list(p):
                    gpsimd.wait_ge(sems["s_prep0" if q == 0 else "s_prep1"],
                                   n_trig[q] + 1)
                    if q == 0:
                        gpsimd.wait_ge(sems["s_cp0"], k + 1)
                    else:
                        if k >= 1:
                            gpsimd.wait_ge(sems["s_cp1"], k)
                        else:
                            gpsimd.wait_ge(sems["s_vinit"], 1)
                        if k >= 3:
                            # gath1 slot WAR vs own h1full store (transitive)
                            gpsimd.wait_ge(sems["s_h1st"], 16 * (k - 2))
                    gpsimd.trigger_dma(1)
                    n_trig[q] += 1
                if p + 1 <= T + 1:
                    for q, k in trig_list(p + 1):
                        prep_bc(q, k)
            assert n_trig == [T, T + 1], n_trig

            # ---- output stage: loads + stores ----
            TG_ = T // NC
            NGR_ = TG_ // 4
            h1f3 = h1full.rearrange("p (s c) -> p s c", c=NN)
            for g in range(NGR_):
                if g >= 2:
                    gpsimd.wait_ge(sems["s_ohsT"], 8 * (g - 1))
                hb = (g % 2) * 8 * 512
                if g == 0:
                    # all T h1full stores done (race-free total)
                    gpsimd.wait_ge(sems["s_h1st"], 16 * T)
                for k in range(NC):
                    with gpsimd.If_eq(rreg, k):
                        t0g = k * TG_ + g * 4
                        for jj in range(8):
                            gpsimd.dma_start(
                                out=h14_sb[:, hb + jj * 512:hb + (jj + 1) * 512]
                                .rearrange("p (s c) -> p s c", c=128),
                                in_=h1f3[:, t0g:t0g + 4, jj * 128:(jj + 1) * 128],
                            ).then_inc(sems["s_og"], 16)
                if g >= 1:
                    for tau in range(4):
                        gpsimd.wait_ge(sems["s_ocp"], 4 * (g - 1) + tau + 1)
                        gpsimd.dma_start(
                            out=out[4 * (g - 1) + tau, :, :],
                            in_=lg_sb[:, tau * VOCAB:(tau + 1) * VOCAB],
                        ).then_inc(sems["s_out"], 16)
            g = NGR_
            for tau in range(4):
                gpsimd.wait_ge(sems["s_ocp"], 4 * (g - 1) + tau + 1)
                gpsimd.dma_start(
                    out=out[4 * (g - 1) + tau, :, :],
                    in_=lg_sb[:, tau * VOCAB:(tau + 1) * VOCAB],
                ).then_inc(sems["s_out"], 16)

    return nc


def _host_prep(inputs, T):
    inp = np.ascontiguousarray(inputs["inputs"][:T]).astype(np.float32)
    emb_W = inputs["emb_W"].astype(np.float32)
    W0 = inputs["lstm_W0"].astype(np.float32)
    b0 = inputs["lstm_b0"].astype(np.float32)
    W1 = inputs["lstm_W1"].astype(np.float32)
    b1 = inputs["lstm_b1"].astype(np.float32)

    flat = inp.reshape(T * B, IND)
    s = np.where(
        (flat[:, VOCAB] == 1.0) & (flat[:, VOCAB + 1] == 0.0), 1.0, -1.0
    ).astype(np.float32)
    inpT_aug = np.zeros((KP, T * B), np.float32)
    inpT_aug[:IND] = flat.T
    inpT_aug[IND] = 1.0
    inpT_aug[IND + 1] = s

    # x-side folded weight: emb @ W0[:512] + flags(b0 row) + rank1(u row)
    Wc = np.zeros((KP, 4 * NN), np.float32)
    Wc[:IND] = emb_W @ W0[:EMB]
    Wc[IND] = b0
    Wc[IND + 1] = W0[EMB:EMB + BIG].sum(axis=0)

    W0h = W0[EMB + BIG:]            # [1024, 4096]
    W1x, W1h = W1[:NN], W1[NN:]

    def gate_cols(W, k):
        return np.concatenate(
            [W[:, base + k * SL:base + (k + 1) * SL] for base in
             (0, NN, 2 * NN, 3 * NN)], axis=1)

    bf = lambda x: np.ascontiguousarray(x).astype(BF16)
    inpT_bf = bf(inpT_aug)
    ow0 = bf(inputs["out_W0"])
    ob0c = np.ascontiguousarray(
        inputs["out_b0"].astype(np.float32).reshape(NC, 128).T
    )
    ow1 = bf(inputs["out_W1"])
    ob1r = bf(inputs["out_b1"].reshape(1, VOCAB))
    iden = bf(np.eye(128, dtype=np.float32))

    in_maps = []
    for k in range(NC):
        in_maps.append({
            "inpT": inpT_bf,
            "wc": bf(gate_cols(Wc, k)),
            "w0h": bf(gate_cols(W0h, k)),
            "w1x": bf(gate_cols(W1x, k)),
            "w1h": bf(gate_cols(W1h, k)),
            "b1r": bf(gate_cols(b1.reshape(1, 4 * NN), k)),
            "ow0": ow0,
            "ob0c": ob0c,
            "ow1": ow1,
            "ob1r": ob1r,
            "iden": iden,
        })
    return in_maps


_CACHE = {}


def run(inputs, T=T_FULL, trace=False):
    if T not in _CACHE:
        nc_new = build(T)
        lower_extended_insts(nc_new)
        _CACHE[T] = nc_new
    nc = _CACHE[T]
    in_maps = _host_prep(inputs, T)
    res = run_bass_kernel_spmd(
        nc, in_maps, core_ids=list(range(NC)), trace=trace
    )
    out = np.concatenate([res.results[k]["out"] for k in range(NC)], axis=0)
    return out, res


def kernel(**inputs):
    out, _ = run(inputs, T=T_FULL)
    return out.astype(np.float32)
